# revision 7
# baseline (speedup 1.0000x reference)
"""Trainium2 Bass kernel for nn_Linear_regression (quadratic regression dot).

out0 = dot(w_lin, x) + dot(w_quad, x*x) + w[2W]
out1 = x[W//2] - out0

Strategy: shard x / w_lin / w_quad along W across 8 cores. Each core
streams its 8MB-per-tensor shard through SBUF in [128, F] fp32 tiles
(double-buffered, raw Bass engine blocks with manual semaphores) and
computes per-partition partial sums with fused vector tensor_tensor_reduce
ops (multiply + reduce + chained accumulate). The x*x term is produced on
the scalar engine (Square activation) so DVE only runs two passes per
element. Per-core output is a [128, 2] tile of partial sums (lin, quad),
reduced on the host along with the two scalar epilogue terms.
"""

import sys
from contextlib import ExitStack

for _p in ("/opt/trn_rl_repo", "/root/.axon_site/_ro/trn_rl_repo"):
    if _p not in sys.path:
        sys.path.append(_p)

import numpy as np

W = 16777216
NCORES = 8
C = W // NCORES          # 2,097,152 elements per core per tensor
P = 128
F = 4096                 # free-dim per tile -> [128, 4096] fp32 = 2 MiB
TILE = P * F             # 524,288 elements
NT = C // TILE           # 4 tiles per tensor per core
NBUF = 2

_cache = {}


def _build():
    import concourse.bass as bass
    from concourse import mybir

    f32 = mybir.dt.float32
    nc = bass.Bass()

    x_d = nc.declare_dram_parameter("x", [NT * P, F], f32, isOutput=False)
    wl_d = nc.declare_dram_parameter("wl", [NT * P, F], f32, isOutput=False)
    wq_d = nc.declare_dram_parameter("wq", [NT * P, F], f32, isOutput=False)
    out_d = nc.declare_dram_parameter("out", [P, 2], f32, isOutput=True)

    mult = mybir.AluOpType.mult
    add = mybir.AluOpType.add

    with ExitStack() as ctx:
        xb = [ctx.enter_context(nc.sbuf_tensor(f"xb{s}", [P, F], f32))
              for s in range(NBUF)]
        wlb = [ctx.enter_context(nc.sbuf_tensor(f"wlb{s}", [P, F], f32))
               for s in range(NBUF)]
        wqb = [ctx.enter_context(nc.sbuf_tensor(f"wqb{s}", [P, F], f32))
               for s in range(NBUF)]
        x2b = [ctx.enter_context(nc.sbuf_tensor(f"x2b{s}", [P, F], f32))
               for s in range(NBUF)]
        prodb = ctx.enter_context(nc.sbuf_tensor("prodb", [P, F], f32))
        accb = ctx.enter_context(nc.sbuf_tensor("accb", [P, 2 * NT], f32))

        sem_in = [ctx.enter_context(nc.semaphore(f"sem_in{s}"))
                  for s in range(NBUF)]
        sem_act = ctx.enter_context(nc.semaphore("sem_act"))
        sem_dve = ctx.enter_context(nc.semaphore("sem_dve"))
        sem_out = ctx.enter_context(nc.semaphore("sem_out"))

        with nc.Block() as block:

            @block.sync
            def _(sync):
                for i in range(NT):
                    s = i % NBUF
                    rows = slice(i * P, (i + 1) * P)
                    if i >= NBUF:
                        # WAR: don't overwrite slot s until compute of
                        # iteration i-NBUF fully consumed it.
                        sync.wait_ge(sem_dve, 2 * (i - NBUF) + 2)
                    sync.dma_start(xb[s][:], x_d[rows, :]).then_inc(sem_in[s], 16)
                    sync.dma_start(wlb[s][:], wl_d[rows, :]).then_inc(sem_in[s], 16)
                    sync.dma_start(wqb[s][:], wq_d[rows, :]).then_inc(sem_in[s], 16)
                sync.wait_ge(sem_dve, 2 * NT)
                sync.dma_start(out_d[:], accb[:]).then_inc(sem_out, 16)
                sync.wait_ge(sem_out, 16)

            @block.scalar
            def _(scalar):
                for i in range(NT):
                    s = i % NBUF
                    k = i // NBUF
                    # whole input trio for this slot landed
                    scalar.wait_ge(sem_in[s], 48 * (k + 1))
                    if i >= NBUF:
                        # WAR on x2b[s]: quad TTR of i-NBUF read it
                        scalar.wait_ge(sem_dve, 2 * (i - NBUF) + 2)
                    scalar.square(out=x2b[s][:], in_=xb[s][:]).then_inc(sem_act, 1)

            @block.vector
            def _(vector):
                for i in range(NT):
                    s = i % NBUF
                    k = i // NBUF
                    vector.wait_ge(sem_in[s], 48 * (k + 1))
                    vector.scalar_tensor_tensor(
                        out=prodb[:], in0=wlb[s][:], scalar=1.0, in1=xb[s][:],
                        op0=mult, op1=mult,
                        accum_out=accb[:, 2 * i:2 * i + 1],
                    ).then_inc(sem_dve, 1)
                    vector.wait_ge(sem_act, i + 1)
                    vector.scalar_tensor_tensor(
                        out=prodb[:], in0=wqb[s][:], scalar=1.0, in1=x2b[s][:],
                        op0=mult, op1=mult,
                        accum_out=accb[:, 2 * i + 1:2 * i + 2],
                    ).then_inc(sem_dve, 1)

    return nc


def _run(inputs: dict, trace: bool = False, tmpdir: str | None = None):
    from concourse.bass_utils import run_bass_kernel_spmd

    if "nc" not in _cache:
        _cache["nc"] = _build()
    nc = _cache["nc"]

    x = np.asarray(inputs["x"], dtype=np.float32)
    w = np.asarray(inputs["weight"], dtype=np.float32)[0]

    xs = x.reshape(NCORES, NT * P, F)
    wls = w[:W].reshape(NCORES, NT * P, F)
    wqs = w[W:2 * W].reshape(NCORES, NT * P, F)

    in_maps = [
        {"x": xs[c], "wl": wls[c], "wq": wqs[c]}
        for c in range(NCORES)
    ]
    res = run_bass_kernel_spmd(
        nc, in_maps, core_ids=list(range(NCORES)),
        trace=trace, tmpdir=tmpdir,
    )

    total = np.float64(0.0)
    for c in range(NCORES):
        total += res.results[c]["out"].astype(np.float64).sum()

    out0 = np.float32(total + np.float64(w[2 * W]))
    out1 = np.float32(x[W // 2]) - out0
    return np.stack([out0, out1]).astype(np.float32), res


def kernel(**inputs) -> np.ndarray:
    out, _ = _run(inputs)
    return out


# revision 10
# speedup vs baseline: 32.2691x; 32.2691x over previous
"""Trainium2 Bass kernel for nn_Linear_regression (quadratic regression dot).

out0 = dot(w_lin, x) + dot(w_quad, x*x) + w[2W]
out1 = x[W//2] - out0

Strategy: shard x / w_lin / w_quad along W across 8 cores. Each core
streams its 8MB-per-tensor shard through SBUF in [128, F] fp32 tiles
(double-buffered, raw Bass engine blocks with manual semaphores) and
computes per-partition partial sums with fused vector tensor_tensor_reduce
ops (multiply + reduce + chained accumulate). The x*x term is produced on
the scalar engine (Square activation) so DVE only runs two passes per
element. Per-core output is a [128, 2] tile of partial sums (lin, quad),
reduced on the host along with the two scalar epilogue terms.
"""

import sys
from contextlib import ExitStack

for _p in ("/opt/trn_rl_repo", "/root/.axon_site/_ro/trn_rl_repo"):
    if _p not in sys.path:
        sys.path.append(_p)

import numpy as np

W = 16777216
NCORES = 8
C = W // NCORES          # 2,097,152 elements per core per tensor
P = 128
F = 4096                 # free-dim per tile -> [128, 4096] fp32 = 2 MiB
TILE = P * F             # 524,288 elements
NT = C // TILE           # 4 tiles per tensor per core
NBUF = 2

_cache = {}


def _build(reps: int = 1):
    import concourse.bass as bass
    from concourse import mybir

    f32 = mybir.dt.float32
    nc = bass.Bass()

    x_d = nc.declare_dram_parameter("x", [NT * P, F], f32, isOutput=False)
    wl_d = nc.declare_dram_parameter("wl", [NT * P, F], f32, isOutput=False)
    wq_d = nc.declare_dram_parameter("wq", [NT * P, F], f32, isOutput=False)
    out_d = nc.declare_dram_parameter("out", [P, 2], f32, isOutput=True)

    mult = mybir.AluOpType.mult
    add = mybir.AluOpType.add

    with ExitStack() as ctx:
        xb = [ctx.enter_context(nc.sbuf_tensor(f"xb{s}", [P, F], f32))
              for s in range(NBUF)]
        wlb = [ctx.enter_context(nc.sbuf_tensor(f"wlb{s}", [P, F], f32))
               for s in range(NBUF)]
        wqb = [ctx.enter_context(nc.sbuf_tensor(f"wqb{s}", [P, F], f32))
               for s in range(NBUF)]
        x2b = [ctx.enter_context(nc.sbuf_tensor(f"x2b{s}", [P, F], f32))
               for s in range(NBUF)]
        prodb = ctx.enter_context(nc.sbuf_tensor("prodb", [P, F], f32))
        accb = ctx.enter_context(nc.sbuf_tensor("accb", [P, 2 * NT], f32))

        sem_in = [ctx.enter_context(nc.semaphore(f"sem_in{s}"))
                  for s in range(NBUF)]
        sem_act = ctx.enter_context(nc.semaphore("sem_act"))
        sem_dve = ctx.enter_context(nc.semaphore("sem_dve"))
        sem_out = ctx.enter_context(nc.semaphore("sem_out"))

        with nc.Block() as block:

            G = NT * reps

            @block.sync
            def _(sync):
                for g in range(G):
                    i = g % NT
                    s = g % NBUF
                    rows = slice(i * P, (i + 1) * P)
                    if g >= NBUF:
                        # WAR: don't overwrite slot s until compute of
                        # iteration g-NBUF fully consumed it.
                        sync.wait_ge(sem_dve, 2 * (g - NBUF) + 2)
                    sync.dma_start(xb[s][:], x_d[rows, :]).then_inc(sem_in[s], 16)
                    sync.dma_start(wlb[s][:], wl_d[rows, :]).then_inc(sem_in[s], 16)
                    sync.dma_start(wqb[s][:], wq_d[rows, :]).then_inc(sem_in[s], 16)
                sync.wait_ge(sem_dve, 2 * G)
                sync.dma_start(out_d[:], accb[:]).then_inc(sem_out, 16)
                sync.wait_ge(sem_out, 16)

            @block.scalar
            def _(scalar):
                for g in range(G):
                    s = g % NBUF
                    k = g // NBUF
                    # whole input trio for this slot landed
                    scalar.wait_ge(sem_in[s], 48 * (k + 1))
                    if g >= NBUF:
                        # WAR on x2b[s]: quad STT of g-NBUF read it
                        scalar.wait_ge(sem_dve, 2 * (g - NBUF) + 2)
                    scalar.square(out=x2b[s][:], in_=xb[s][:]).then_inc(sem_act, 1)

            @block.vector
            def _(vector):
                for g in range(G):
                    i = g % NT
                    s = g % NBUF
                    k = g // NBUF
                    vector.wait_ge(sem_in[s], 48 * (k + 1))
                    vector.scalar_tensor_tensor(
                        out=prodb[:], in0=wlb[s][:], scalar=1.0, in1=xb[s][:],
                        op0=mult, op1=mult,
                        accum_out=accb[:, 2 * i:2 * i + 1],
                    ).then_inc(sem_dve, 1)
                    vector.wait_ge(sem_act, g + 1)
                    vector.scalar_tensor_tensor(
                        out=prodb[:], in0=wqb[s][:], scalar=1.0, in1=x2b[s][:],
                        op0=mult, op1=mult,
                        accum_out=accb[:, 2 * i + 1:2 * i + 2],
                    ).then_inc(sem_dve, 1)

    return nc


def _run(inputs: dict, trace: bool = False, tmpdir: str | None = None):
    from concourse.bass_utils import run_bass_kernel_spmd

    if "nc" not in _cache:
        _cache["nc"] = _build(reps=1)
    nc = _cache["nc"]

    x = np.asarray(inputs["x"], dtype=np.float32)
    w = np.asarray(inputs["weight"], dtype=np.float32)[0]

    xs = x.reshape(NCORES, NT * P, F)
    wls = w[:W].reshape(NCORES, NT * P, F)
    wqs = w[W:2 * W].reshape(NCORES, NT * P, F)

    in_maps = [
        {"x": xs[c], "wl": wls[c], "wq": wqs[c]}
        for c in range(NCORES)
    ]
    res = run_bass_kernel_spmd(
        nc, in_maps, core_ids=list(range(NCORES)),
        trace=trace, tmpdir=tmpdir,
    )

    total = np.float64(0.0)
    for c in range(NCORES):
        total += res.results[c]["out"].astype(np.float64).sum()

    out0 = np.float32(total + np.float64(w[2 * W]))
    out1 = np.float32(x[W // 2]) - out0
    return np.stack([out0, out1]).astype(np.float32), res


def kernel(**inputs) -> np.ndarray:
    out, _ = _run(inputs)
    return out
